# revision 7
# baseline (speedup 1.0000x reference)
"""DLRM (nn_DLRM_Net) Trainium2 kernel.

Strategy: data-parallel over the batch on 8 NeuronCores. Each core gets the
full embedding tables (bf16, flattened to [26*200000, 64]) plus its 4096-sample
batch shard. Per core: bottom MLP (PE, bf16), embedding gather (indirect DMA,
128B rows), per-sample 27x27 Gram interaction (PE transposes + per-sample
matmuls), pair extraction (SBUF->SBUF DMAs), top MLP (PE), sigmoid (ACT).
"""

import sys

sys.path.insert(0, "/opt/trn_rl_repo")

import numpy as np
import ml_dtypes

import concourse.bass as bass
import concourse.mybir as mybir
import concourse.tile as tile
from concourse.bass import IndirectOffsetOnAxis, ds, ts
from concourse.bass_utils import run_bass_kernel_spmd
from concourse.masks import make_identity
from concourse.vector_clock import ScopedClock

BF16 = mybir.dt.bfloat16
F32 = mybir.dt.float32
I32 = mybir.dt.int32

B = 32768
NT = 26          # tables
NR = 200000      # rows per table
D = 64           # embedding dim
ND = 13          # dense features
NF = NT + 1      # feature vectors per sample (x_bot + 26 embeddings)
NPAIR = NF * (NF - 1) // 2  # 351
N_CORES = 8
BC = B // N_CORES  # 4096 per core


def split_excess_waits(nc, cap=1):
    """Walrus in this container rejects instructions carrying more than one
    sync-wait. Hoist extra waits into standalone InstEventSemaphore
    instructions inserted just before the carrying instruction (same engine,
    so sequencer order preserves the stall semantics)."""
    n_split = 0
    for f in nc.m.functions:
        for bb in f.blocks:
            insts = bb.instructions
            out = []
            changed = False
            for ins in insts:
                si = ins.sync_info
                if si is not None and len(si.on_wait) > cap:
                    extra = list(si.on_wait[cap:])
                    del si.on_wait[cap:]
                    for w in extra:
                        ev = mybir.InstEventSemaphore(
                            name=f"{ins.name}-xw{n_split}", ins=[], outs=[])
                        ev.engine = ins.engine
                        ev.sync_info = mybir.SyncInfo(on_wait=[w], on_update=[])
                        try:
                            nc.register_instruction(ev, overwrite=True)
                        except Exception:
                            pass
                        out.append(ev)
                        n_split += 1
                    changed = True
                out.append(ins)
            if changed:
                if hasattr(insts, "clear"):
                    insts.clear()
                    insts.extend(out)
                else:
                    bb.instructions = out
    return n_split


def pair_segments():
    """Zflat row placement: pair p=(li,lj), j<i, li-major (jnp.tril_indices
    order). Row in R = 64 + p. Returns per-li DMA segments, split at 128-row
    k-tile boundaries: (li, src_row0, dst_row_global, nrows)."""
    segs = []
    for li in range(1, NF):
        base = 64 + li * (li - 1) // 2  # global R-row of pair (li, 0)
        j0 = 0
        while j0 < li:
            r0 = base + j0
            # rows r0 .. base+li-1, clipped to the 128-boundary
            nmax = li - j0
            room = 128 - (r0 % 128)
            n = min(nmax, room)
            segs.append((li, j0, r0, n))
            j0 += n
    return segs


PAIR_SEGS = pair_segments()


def build_kernel(bc=BC, n_tab_rows=NT * NR, s_chunk=512):
    """Build the per-core Bass kernel. bc = per-core batch, s_chunk must
    divide bc and be a multiple of 256."""
    assert bc % s_chunk == 0 and s_chunk % 256 == 0
    nch = bc // s_chunk
    S = s_chunk
    SH = S // 2           # samples per TT partition-half
    NBT = S // 128        # 128-sample gather tiles per chunk
    GS = 18               # grams packed per PSUM bank (18*27=486 <= 512)

    nc = bass.Bass(trn_type="TRN2")

    tabs = nc.dram_tensor("tabs", [n_tab_rows, D], BF16, kind="ExternalInput")
    gidx = nc.dram_tensor("gidx", [bc, NT], I32, kind="ExternalInput")
    dxt = nc.dram_tensor("dxt", [16, bc], BF16, kind="ExternalInput")
    w0 = nc.dram_tensor("w0", [16, 512], BF16, kind="ExternalInput")
    b0 = nc.dram_tensor("b0", [128, 4], F32, kind="ExternalInput")
    w1 = nc.dram_tensor("w1", [128, 4, 256], BF16, kind="ExternalInput")
    b1 = nc.dram_tensor("b1", [128, 2], F32, kind="ExternalInput")
    w2 = nc.dram_tensor("w2", [128, 2, 64], BF16, kind="ExternalInput")
    b2 = nc.dram_tensor("b2", [128, 1], F32, kind="ExternalInput")
    tw0 = nc.dram_tensor("tw0", [128, 4, 512], BF16, kind="ExternalInput")
    tb0 = nc.dram_tensor("tb0", [128, 4], F32, kind="ExternalInput")
    tw1 = nc.dram_tensor("tw1", [128, 4, 256], BF16, kind="ExternalInput")
    tb1 = nc.dram_tensor("tb1", [128, 2], F32, kind="ExternalInput")
    tw2 = nc.dram_tensor("tw2", [128, 2, 1], BF16, kind="ExternalInput")
    tb2 = nc.dram_tensor("tb2", [1, 1], F32, kind="ExternalInput")
    out = nc.dram_tensor("out", [1, bc], F32, kind="ExternalOutput")

    with tile.TileContext(nc) as tc:
        with (
            tc.tile_pool(name="const", bufs=1) as cpool,
            tc.tile_pool(name="work", bufs=2) as wpool,
            tc.tile_pool(name="gath", bufs=8) as gpool,
            tc.tile_pool(name="big", bufs=2) as bpool,
            tc.tile_pool(name="psmm", bufs=2, space="PSUM") as ps_mm,
            tc.tile_pool(name="pstr", bufs=2, space="PSUM") as ps_tr,
            tc.tile_pool(name="psgr", bufs=2, space="PSUM") as ps_gr,
        ):
            # ---- persistent loads ----
            ident = cpool.tile([128, 128], BF16)
            make_identity(nc, ident[:])

            w0_sb = cpool.tile([16, 512], BF16)
            nc.sync.dma_start(w0_sb[:], w0[:, :])
            w1_sb = cpool.tile([128, 4, 256], BF16)
            nc.sync.dma_start(w1_sb[:], w1[:, :, :])
            w2_sb = cpool.tile([128, 2, 64], BF16)
            nc.sync.dma_start(w2_sb[:], w2[:, :, :])
            tw0_sb = cpool.tile([128, 4, 512], BF16)
            nc.sync.dma_start(tw0_sb[:], tw0[:, :, :])
            tw1_sb = cpool.tile([128, 4, 256], BF16)
            nc.sync.dma_start(tw1_sb[:], tw1[:, :, :])
            tw2_sb = cpool.tile([128, 2, 1], BF16)
            nc.sync.dma_start(tw2_sb[:], tw2[:, :, :])
            b0_sb = cpool.tile([128, 4], F32)
            nc.sync.dma_start(b0_sb[:], b0[:, :])
            b1_sb = cpool.tile([128, 2], F32)
            nc.sync.dma_start(b1_sb[:], b1[:, :])
            b2_sb = cpool.tile([128, 1], F32)
            nc.sync.dma_start(b2_sb[:], b2[:, :])
            tb0_sb = cpool.tile([128, 4], F32)
            nc.sync.dma_start(tb0_sb[:], tb0[:, :])
            tb1_sb = cpool.tile([128, 2], F32)
            nc.sync.dma_start(tb1_sb[:], tb1[:, :])
            tb2_sb = cpool.tile([1, 1], F32)
            nc.sync.dma_start(tb2_sb[:], tb2[:, :])

            dxt_sb = cpool.tile([16, bc], BF16)
            nc.sync.dma_start(dxt_sb[:], dxt[:, :])
            # indices: [bc, NT] -> [128, bc//128, NT]
            idx_sb = cpool.tile([128, bc // 128, NT], I32)
            nc.sync.dma_start(idx_sb[:], gidx.rearrange("(t p) k -> p t k", p=128))

            for c in range(nch):
                cs = ds(c * S, S)

                # ---- bottom MLP for this chunk (feature-major) ----
                h0 = wpool.tile([128, 4, S], BF16, tag="h0")
                for mt in range(4):
                    ps = ps_mm.tile([128, 512], F32, tag="ps")
                    nc.tensor.matmul(ps[:, :S], w0_sb[:, ts(mt, 128)],
                                     dxt_sb[:, cs], start=True, stop=True)
                    nc.scalar.activation(h0[:, mt, :], ps[:, :S],
                                         mybir.ActivationFunctionType.Relu,
                                         bias=b0_sb[:, mt, None])
                h1 = wpool.tile([128, 2, S], BF16, tag="h1")
                for mt in range(2):
                    ps = ps_mm.tile([128, 512], F32, tag="ps")
                    for k in range(4):
                        nc.tensor.matmul(ps[:, :S], w1_sb[:, k, ts(mt, 128)],
                                         h0[:, k, :], start=(k == 0), stop=(k == 3))
                    nc.scalar.activation(h1[:, mt, :], ps[:, :S],
                                         mybir.ActivationFunctionType.Relu,
                                         bias=b1_sb[:, mt, None])
                # x_bot, duplicated on both partition halves
                psx = ps_mm.tile([128, 512], F32, tag="ps")
                for k in range(2):
                    nc.tensor.matmul(psx[0:64, :S], w2_sb[:, k, :], h1[:, k, :],
                                     start=(k == 0), stop=(k == 1))
                for k in range(2):
                    nc.tensor.matmul(psx[64:128, :S], w2_sb[:, k, :], h1[:, k, :],
                                     start=(k == 0), stop=(k == 1),
                                     tile_position=(0, 64))
                xbd = wpool.tile([128, S], BF16, tag="xbd")
                nc.scalar.activation(xbd[:, :], psx[:, :S],
                                     mybir.ActivationFunctionType.Relu,
                                     bias=b2_sb[:, 0, None])

                # ---- gather + transpose into TT [128, NF, SH] ----
                TT = bpool.tile([128, NF, SH], BF16, tag="TT")
                nc.vector.tensor_copy(TT[0:64, 0, :], xbd[0:64, 0:SH])
                nc.vector.tensor_copy(TT[64:128, 0, :], xbd[64:128, SH:S])
                for bt in range(NBT):
                    gE = gpool.tile([128, NT * D], BF16, tag="gE")
                    nc.gpsimd.indirect_dma_start(
                        out=gE[:, :], out_offset=None, in_=tabs[:, :],
                        in_offset=IndirectOffsetOnAxis(
                            ap=idx_sb[:, c * NBT + bt, :], axis=0))
                    half = 0 if bt < NBT // 2 else 64
                    colb = (bt % (NBT // 2)) * 128
                    for n in range(1, NF):
                        tp = ps_tr.tile([128, 128], BF16, tag="tp")
                        nc.tensor.transpose(
                            out=tp[half:half + 64, :],
                            in_=gE[:, ds((n - 1) * D, D)],
                            identity=ident[:])
                        nc.vector.tensor_copy(
                            TT[half:half + 64, n, ds(colb, 128)],
                            tp[half:half + 64, :])

                # ---- per-sample Gram -> Gsb [27, 27, S] (bf16) ----
                Gsb = bpool.tile([NF, NF, S], BF16, tag="Gsb")
                ngrp = (SH + GS - 1) // GS
                for g in range(ngrp):
                    q0 = g * GS
                    qn = min(GS, SH - q0)
                    psa = ps_gr.tile([NF, GS * NF], F32, tag="psa")
                    psb = ps_gr.tile([NF, GS * NF], F32, tag="psb")
                    for j in range(qn):
                        q = q0 + j
                        nc.tensor.matmul(psa[:, ds(j * NF, NF)],
                                         TT[0:64, :, q], TT[0:64, :, q],
                                         start=True, stop=True)
                        nc.tensor.matmul(psb[:, ds(j * NF, NF)],
                                         TT[64:128, :, q], TT[64:128, :, q],
                                         start=True, stop=True)
                    # copy to Gsb: A-half samples q0..q0+qn, B-half SH+q0..
                    nc.vector.tensor_copy(
                        Gsb[:, :, ds(q0, qn)].rearrange("p n s -> p s n"),
                        psa[:, :qn * NF].rearrange("p (s n) -> p s n", n=NF))
                    nc.vector.tensor_copy(
                        Gsb[:, :, ds(SH + q0, qn)].rearrange("p n s -> p s n"),
                        psb[:, :qn * NF].rearrange("p (s n) -> p s n", n=NF))

                # ---- assemble R [512 rows, S] as 4 k-tiles ----
                Rsb = bpool.tile([128, 4, S], BF16, tag="Rsb")
                nc.vector.tensor_copy(Rsb[0:64, 0, :], xbd[0:64, :])
                nc.any.memset(Rsb[:, 3, :], 0.0)
                for (li, j0, r0, nrows) in PAIR_SEGS:
                    nc.sync.dma_start(
                        Rsb[r0 % 128:r0 % 128 + nrows, r0 // 128, :],
                        Gsb[j0:j0 + nrows, li, :])

                # ---- top MLP ----
                th0 = wpool.tile([128, 4, S], BF16, tag="th0")
                for mt in range(4):
                    ps = ps_mm.tile([128, 512], F32, tag="ps")
                    for k in range(4):
                        nc.tensor.matmul(ps[:, :S], tw0_sb[:, k, ts(mt, 128)],
                                         Rsb[:, k, :], start=(k == 0), stop=(k == 3))
                    nc.scalar.activation(th0[:, mt, :], ps[:, :S],
                                         mybir.ActivationFunctionType.Relu,
                                         bias=tb0_sb[:, mt, None])
                th1 = wpool.tile([128, 2, S], BF16, tag="th1")
                for mt in range(2):
                    ps = ps_mm.tile([128, 512], F32, tag="ps")
                    for k in range(4):
                        nc.tensor.matmul(ps[:, :S], tw1_sb[:, k, ts(mt, 128)],
                                         th0[:, k, :], start=(k == 0), stop=(k == 3))
                    nc.scalar.activation(th1[:, mt, :], ps[:, :S],
                                         mybir.ActivationFunctionType.Relu,
                                         bias=tb1_sb[:, mt, None])
                ps2 = ps_mm.tile([128, 512], F32, tag="ps")
                for k in range(2):
                    nc.tensor.matmul(ps2[0:1, :S], tw2_sb[:, k, :], th1[:, k, :],
                                     start=(k == 0), stop=(k == 1))
                osb = wpool.tile([1, S], F32, tag="osb")
                nc.scalar.activation(osb[:, :], ps2[0:1, :S],
                                     mybir.ActivationFunctionType.Sigmoid,
                                     bias=tb2_sb[:, 0, None])
                nc.sync.dma_start(out[0:1, cs], osb[:, :])

    n = split_excess_waits(nc)
    print(f"split_excess_waits: {n} hoisted", flush=True)
    return nc


def prep_host_inputs(dense_x, indices, emb_tables,
                     bw0, bb0, bw1, bb1, bw2, bb2,
                     tw0, tb0, tw1, tb1, tw2, tb2, bc=BC, n_cores=N_CORES):
    tw0_, tb0_, tw1_, tb1_, tw2_, tb2_ = tw0, tb0, tw1, tb1, tw2, tb2
    """Build the shared (replicated) and per-core input arrays."""
    bf = ml_dtypes.bfloat16
    nt, nr, d = emb_tables.shape
    tabs = np.ascontiguousarray(emb_tables.reshape(nt * nr, d)).astype(bf)

    def pad(a, shape):
        o = np.zeros(shape, np.float32)
        o[tuple(slice(0, s) for s in a.shape)] = a
        return o

    w0h = pad(np.asarray(bw0, np.float32).T, (16, 512)).astype(bf)
    w1h = pad(np.asarray(bw1, np.float32).T, (512, 256)).reshape(4, 128, 256)
    w1h = np.ascontiguousarray(w1h.transpose(1, 0, 2)).astype(bf)
    w2h = pad(np.asarray(bw2, np.float32).T, (256, 64)).reshape(2, 128, 64)
    w2h = np.ascontiguousarray(w2h.transpose(1, 0, 2)).astype(bf)
    tw0h = pad(np.asarray(tw0_, np.float32).T, (512, 512)).reshape(4, 128, 512)
    tw0h = np.ascontiguousarray(tw0h.transpose(1, 0, 2)).astype(bf)
    tw1h = pad(np.asarray(tw1_, np.float32).T, (512, 256)).reshape(4, 128, 256)
    tw1h = np.ascontiguousarray(tw1h.transpose(1, 0, 2)).astype(bf)
    tw2h = pad(np.asarray(tw2_, np.float32).T, (256, 1)).reshape(2, 128, 1)
    tw2h = np.ascontiguousarray(tw2h.transpose(1, 0, 2)).astype(bf)

    b0h = pad(np.asarray(bb0, np.float32), (512,)).reshape(4, 128).T.copy()
    b1h = pad(np.asarray(bb1, np.float32), (256,)).reshape(2, 128).T.copy()
    b2h = np.tile(pad(np.asarray(bb2, np.float32), (64,)), 2).reshape(128, 1).copy()
    tb0h = pad(np.asarray(tb0_, np.float32), (512,)).reshape(4, 128).T.copy()
    tb1h = pad(np.asarray(tb1_, np.float32), (256,)).reshape(2, 128).T.copy()
    tb2h = np.asarray(tb2_, np.float32).reshape(1, 1).copy()

    shared = dict(tabs=tabs, w0=w0h, b0=b0h, w1=w1h, b1=b1h, w2=w2h, b2=b2h,
                  tw0=tw0h, tb0=tb0h, tw1=tw1h, tb1=tb1h, tw2=tw2h, tb2=tb2h)

    # global row index = table*nr + idx; per-core sample-major [bc, NT]
    gflat = (np.arange(nt, dtype=np.int64)[:, None] * nr
             + np.asarray(indices, np.int64)).astype(np.int32)  # [NT, B]
    in_maps = []
    for c in range(n_cores):
        sl = slice(c * bc, (c + 1) * bc)
        gi = np.ascontiguousarray(gflat[:, sl].T)  # [bc, NT]
        dx = np.zeros((16, bc), np.float32)
        dx[:ND] = np.asarray(dense_x, np.float32)[sl].T
        in_maps.append(dict(shared, gidx=gi, dxt=dx.astype(bf)))
    return in_maps


_NC_CACHE = {}


def kernel(**inputs):
    key = "full"
    if key not in _NC_CACHE:
        _NC_CACHE[key] = build_kernel()
    nc = _NC_CACHE[key]
    in_maps = prep_host_inputs(**inputs)
    res = run_bass_kernel_spmd(nc, in_maps, core_ids=list(range(N_CORES)))
    outs = [res.results[c]["out"].reshape(BC) for c in range(N_CORES)]
    return np.concatenate(outs).reshape(B, 1).astype(np.float32)


# revision 11
# speedup vs baseline: 1.6144x; 1.6144x over previous
"""DLRM (nn_DLRM_Net) Trainium2 kernel.

Strategy: data-parallel over the batch on 8 NeuronCores. Each core gets the
full embedding tables (bf16, flattened to [26*200000, 64]) plus its 4096-sample
batch shard. Per core: bottom MLP (PE, bf16), embedding gather (indirect DMA,
128B rows), per-sample 27x27 Gram interaction (PE transposes + per-sample
matmuls), pair extraction (SBUF->SBUF DMAs), top MLP (PE), sigmoid (ACT).
"""

import sys

sys.path.insert(0, "/opt/trn_rl_repo")

import numpy as np
import ml_dtypes

import concourse.bass as bass
import concourse.mybir as mybir
import concourse.tile as tile
from concourse.bass import IndirectOffsetOnAxis, ds, ts
from concourse.bass_utils import run_bass_kernel_spmd
from concourse.masks import make_identity
from concourse.vector_clock import ScopedClock

BF16 = mybir.dt.bfloat16
F32 = mybir.dt.float32
I32 = mybir.dt.int32

B = 32768
NT = 26          # tables
NR = 200000      # rows per table
D = 64           # embedding dim
ND = 13          # dense features
NF = NT + 1      # feature vectors per sample (x_bot + 26 embeddings)
NPAIR = NF * (NF - 1) // 2  # 351
N_CORES = 8
BC = B // N_CORES  # 4096 per core


def split_excess_waits(nc, cap=1):
    """Walrus in this container rejects instructions carrying more than one
    sync-wait. Hoist extra waits into standalone InstEventSemaphore
    instructions inserted just before the carrying instruction (same engine,
    so sequencer order preserves the stall semantics)."""
    n_split = 0
    for f in nc.m.functions:
        for bb in f.blocks:
            insts = bb.instructions
            out = []
            changed = False
            for ins in insts:
                si = ins.sync_info
                if si is not None and len(si.on_wait) > cap:
                    extra = list(si.on_wait[cap:])
                    del si.on_wait[cap:]
                    for w in extra:
                        ev = mybir.InstEventSemaphore(
                            name=f"{ins.name}-xw{n_split}", ins=[], outs=[])
                        ev.engine = ins.engine
                        ev.sync_info = mybir.SyncInfo(on_wait=[w], on_update=[])
                        try:
                            nc.register_instruction(ev, overwrite=True)
                        except Exception:
                            pass
                        out.append(ev)
                        n_split += 1
                    changed = True
                out.append(ins)
            if changed:
                if hasattr(insts, "clear"):
                    insts.clear()
                    insts.extend(out)
                else:
                    bb.instructions = out
    return n_split


def pair_segments():
    """Zflat row placement: pair p=(li,lj), j<i, li-major (jnp.tril_indices
    order). Row in R = 64 + p. Returns per-li DMA segments, split at 128-row
    k-tile boundaries: (li, src_row0, dst_row_global, nrows)."""
    segs = []
    for li in range(1, NF):
        base = 64 + li * (li - 1) // 2  # global R-row of pair (li, 0)
        j0 = 0
        while j0 < li:
            r0 = base + j0
            # rows r0 .. base+li-1, clipped to the 128-boundary
            nmax = li - j0
            room = 128 - (r0 % 128)
            n = min(nmax, room)
            segs.append((li, j0, r0, n))
            j0 += n
    return segs


PAIR_SEGS = pair_segments()


def build_kernel(bc=BC, n_tab_rows=NT * NR, s_chunk=512):
    """Build the per-core Bass kernel. bc = per-core batch, s_chunk must
    divide bc and be a multiple of 256."""
    assert bc % s_chunk == 0 and s_chunk % 256 == 0
    nch = bc // s_chunk
    S = s_chunk
    SH = S // 2           # samples per TT partition-half
    NBT = S // 128        # 128-sample gather tiles per chunk
    GS = 16               # grams per PSUM tile ([27, 27, 16] f32 = 1728B/bank)

    nc = bass.Bass(trn_type="TRN2")

    tabs = nc.dram_tensor("tabs", [n_tab_rows, D], BF16, kind="ExternalInput")
    gidx = nc.dram_tensor("gidx", [bc, NT], I32, kind="ExternalInput")
    dxt = nc.dram_tensor("dxt", [16, bc], BF16, kind="ExternalInput")
    w0 = nc.dram_tensor("w0", [16, 512], BF16, kind="ExternalInput")
    b0 = nc.dram_tensor("b0", [128, 4], F32, kind="ExternalInput")
    w1 = nc.dram_tensor("w1", [128, 4, 256], BF16, kind="ExternalInput")
    b1 = nc.dram_tensor("b1", [128, 2], F32, kind="ExternalInput")
    w2 = nc.dram_tensor("w2", [128, 2, 64], BF16, kind="ExternalInput")
    b2 = nc.dram_tensor("b2", [128, 1], F32, kind="ExternalInput")
    tw0 = nc.dram_tensor("tw0", [128, 4, 512], BF16, kind="ExternalInput")
    tb0 = nc.dram_tensor("tb0", [128, 4], F32, kind="ExternalInput")
    tw1 = nc.dram_tensor("tw1", [128, 4, 256], BF16, kind="ExternalInput")
    tb1 = nc.dram_tensor("tb1", [128, 2], F32, kind="ExternalInput")
    tw2 = nc.dram_tensor("tw2", [128, 2, 1], BF16, kind="ExternalInput")
    tb2 = nc.dram_tensor("tb2", [1, 1], F32, kind="ExternalInput")
    out = nc.dram_tensor("out", [1, bc], F32, kind="ExternalOutput")

    with tile.TileContext(nc) as tc:
        with (
            tc.tile_pool(name="const", bufs=1) as cpool,
            tc.tile_pool(name="work", bufs=2) as wpool,
            tc.tile_pool(name="gath", bufs=8) as gpool,
            tc.tile_pool(name="big", bufs=2) as bpool,
            tc.tile_pool(name="psmm", bufs=2, space="PSUM") as ps_mm,
            tc.tile_pool(name="pstr", bufs=2, space="PSUM") as ps_tr,
            tc.tile_pool(name="psgr", bufs=2, space="PSUM") as ps_gr,
        ):
            # ---- persistent loads ----
            ident = cpool.tile([128, 128], BF16)
            make_identity(nc, ident[:])

            w0_sb = cpool.tile([16, 512], BF16)
            nc.sync.dma_start(w0_sb[:], w0[:, :])
            w1_sb = cpool.tile([128, 4, 256], BF16)
            nc.sync.dma_start(w1_sb[:], w1[:, :, :])
            w2_sb = cpool.tile([128, 2, 64], BF16)
            nc.sync.dma_start(w2_sb[:], w2[:, :, :])
            tw0_sb = cpool.tile([128, 4, 512], BF16)
            nc.sync.dma_start(tw0_sb[:], tw0[:, :, :])
            tw1_sb = cpool.tile([128, 4, 256], BF16)
            nc.sync.dma_start(tw1_sb[:], tw1[:, :, :])
            tw2_sb = cpool.tile([128, 2, 1], BF16)
            nc.sync.dma_start(tw2_sb[:], tw2[:, :, :])
            b0_sb = cpool.tile([128, 4], F32)
            nc.sync.dma_start(b0_sb[:], b0[:, :])
            b1_sb = cpool.tile([128, 2], F32)
            nc.sync.dma_start(b1_sb[:], b1[:, :])
            b2_sb = cpool.tile([128, 1], F32)
            nc.sync.dma_start(b2_sb[:], b2[:, :])
            tb0_sb = cpool.tile([128, 4], F32)
            nc.sync.dma_start(tb0_sb[:], tb0[:, :])
            tb1_sb = cpool.tile([128, 2], F32)
            nc.sync.dma_start(tb1_sb[:], tb1[:, :])
            tb2_sb = cpool.tile([1, 1], F32)
            nc.sync.dma_start(tb2_sb[:], tb2[:, :])

            dxt_sb = cpool.tile([16, bc], BF16)
            nc.sync.dma_start(dxt_sb[:], dxt[:, :])
            # indices: [bc, NT] -> [128, bc//128, NT]
            idx_sb = cpool.tile([128, bc // 128, NT], I32)
            nc.sync.dma_start(idx_sb[:], gidx.rearrange("(t p) k -> p t k", p=128))

            for c in range(nch):
                cs = ds(c * S, S)

                # ---- bottom MLP for this chunk (feature-major) ----
                h0 = wpool.tile([128, 4, S], BF16, tag="h0")
                for mt in range(4):
                    ps = ps_mm.tile([128, 512], F32, tag="ps")
                    nc.tensor.matmul(ps[:, :S], w0_sb[:, ts(mt, 128)],
                                     dxt_sb[:, cs], start=True, stop=True)
                    nc.scalar.activation(h0[:, mt, :], ps[:, :S],
                                         mybir.ActivationFunctionType.Relu,
                                         bias=b0_sb[:, mt, None])
                h1 = wpool.tile([128, 2, S], BF16, tag="h1")
                for mt in range(2):
                    ps = ps_mm.tile([128, 512], F32, tag="ps")
                    for k in range(4):
                        nc.tensor.matmul(ps[:, :S], w1_sb[:, k, ts(mt, 128)],
                                         h0[:, k, :], start=(k == 0), stop=(k == 3))
                    nc.scalar.activation(h1[:, mt, :], ps[:, :S],
                                         mybir.ActivationFunctionType.Relu,
                                         bias=b1_sb[:, mt, None])
                # x_bot, duplicated on both partition halves
                psx = ps_mm.tile([128, 512], F32, tag="ps")
                for k in range(2):
                    nc.tensor.matmul(psx[0:64, :S], w2_sb[:, k, :], h1[:, k, :],
                                     start=(k == 0), stop=(k == 1))
                for k in range(2):
                    nc.tensor.matmul(psx[64:128, :S], w2_sb[:, k, :], h1[:, k, :],
                                     start=(k == 0), stop=(k == 1),
                                     tile_position=(0, 64))
                xbd = wpool.tile([128, S], BF16, tag="xbd")
                nc.scalar.activation(xbd[:, :], psx[:, :S],
                                     mybir.ActivationFunctionType.Relu,
                                     bias=b2_sb[:, 0, None])

                # ---- gather + transpose into TT [128, NF, SH] ----
                # A-half samples (chunk 0..SH) on partitions 0:64, B-half on
                # 64:128. One psum tile holds the same feature for an A-tile
                # (rows 0:64) and its B-tile partner (rows 64:128) so a single
                # [128,128] DVE copy moves both.
                TT = bpool.tile([128, NF, SH], BF16, tag="TT")
                nc.vector.tensor_copy(TT[0:64, 0, :], xbd[0:64, 0:SH])
                nc.vector.tensor_copy(TT[64:128, 0, :], xbd[64:128, SH:S])
                NBH = NBT // 2
                gEs = []
                for bt in range(NBT):
                    gE = gpool.tile([128, NT * D], BF16, tag="gE")
                    nc.gpsimd.indirect_dma_start(
                        out=gE[:, :], out_offset=None, in_=tabs[:, :],
                        in_offset=IndirectOffsetOnAxis(
                            ap=idx_sb[:, c * NBT + bt, :], axis=0))
                    gEs.append(gE)
                for bt in range(NBH):
                    colb = bt * 128
                    for n in range(1, NF):
                        tp = ps_tr.tile([128, 128], BF16, tag="tp")
                        nc.tensor.transpose(
                            out=tp[0:64, :],
                            in_=gEs[bt][:, ds((n - 1) * D, D)],
                            identity=ident[:])
                        nc.tensor.transpose(
                            out=tp[64:128, :],
                            in_=gEs[NBH + bt][:, ds((n - 1) * D, D)],
                            identity=ident[:])
                        nc.vector.tensor_copy(
                            TT[:, n, ds(colb, 128)], tp[:, :])

                # ---- per-sample Gram -> Gsb [27, 27, S] (bf16) ----
                # psum layout [27, 27, GS]: matmul j writes its 27 output
                # columns at stride GS (one f32 per column per partition), so
                # the psum->Gsb cast is (n outer, sample inner) with a
                # contiguous GS-sample inner run on both sides.
                Gsb = bpool.tile([NF, NF, S], BF16, tag="Gsb")
                ngrp = SH // GS
                for g in range(ngrp):
                    q0 = g * GS
                    psa = ps_gr.tile([NF, NF, GS], F32, tag="psa")
                    psb = ps_gr.tile([NF, NF, GS], F32, tag="psb")
                    for j in range(GS):
                        q = q0 + j
                        nc.tensor.matmul(psa[:, :, j],
                                         TT[0:64, :, q], TT[0:64, :, q],
                                         start=True, stop=True)
                        nc.tensor.matmul(psb[:, :, j],
                                         TT[64:128, :, q], TT[64:128, :, q],
                                         start=True, stop=True)
                    nc.vector.tensor_copy(Gsb[:, :, ds(q0, GS)], psa[:, :, :])
                    nc.vector.tensor_copy(Gsb[:, :, ds(SH + q0, GS)],
                                          psb[:, :, :])

                # ---- assemble R [512 rows, S] as 4 k-tiles ----
                Rsb = bpool.tile([128, 4, S], BF16, tag="Rsb")
                nc.vector.tensor_copy(Rsb[0:64, 0, :], xbd[0:64, :])
                nc.any.memset(Rsb[:, 3, :], 0.0)
                for si, (li, j0, r0, nrows) in enumerate(PAIR_SEGS):
                    eng = nc.sync if si % 2 == 0 else nc.scalar
                    eng.dma_start(
                        Rsb[r0 % 128:r0 % 128 + nrows, r0 // 128, :],
                        Gsb[j0:j0 + nrows, li, :])

                # ---- top MLP ----
                th0 = wpool.tile([128, 4, S], BF16, tag="th0")
                for mt in range(4):
                    ps = ps_mm.tile([128, 512], F32, tag="ps")
                    for k in range(4):
                        nc.tensor.matmul(ps[:, :S], tw0_sb[:, k, ts(mt, 128)],
                                         Rsb[:, k, :], start=(k == 0), stop=(k == 3))
                    nc.scalar.activation(th0[:, mt, :], ps[:, :S],
                                         mybir.ActivationFunctionType.Relu,
                                         bias=tb0_sb[:, mt, None])
                th1 = wpool.tile([128, 2, S], BF16, tag="th1")
                for mt in range(2):
                    ps = ps_mm.tile([128, 512], F32, tag="ps")
                    for k in range(4):
                        nc.tensor.matmul(ps[:, :S], tw1_sb[:, k, ts(mt, 128)],
                                         th0[:, k, :], start=(k == 0), stop=(k == 3))
                    nc.scalar.activation(th1[:, mt, :], ps[:, :S],
                                         mybir.ActivationFunctionType.Relu,
                                         bias=tb1_sb[:, mt, None])
                ps2 = ps_mm.tile([128, 512], F32, tag="ps")
                for k in range(2):
                    nc.tensor.matmul(ps2[0:1, :S], tw2_sb[:, k, :], th1[:, k, :],
                                     start=(k == 0), stop=(k == 1))
                osb = wpool.tile([1, S], F32, tag="osb")
                nc.scalar.activation(osb[:, :], ps2[0:1, :S],
                                     mybir.ActivationFunctionType.Sigmoid,
                                     bias=tb2_sb[:, 0, None])
                nc.sync.dma_start(out[0:1, cs], osb[:, :])

    n = split_excess_waits(nc)
    print(f"split_excess_waits: {n} hoisted", flush=True)
    return nc


def prep_host_inputs(dense_x, indices, emb_tables,
                     bw0, bb0, bw1, bb1, bw2, bb2,
                     tw0, tb0, tw1, tb1, tw2, tb2, bc=BC, n_cores=N_CORES):
    tw0_, tb0_, tw1_, tb1_, tw2_, tb2_ = tw0, tb0, tw1, tb1, tw2, tb2
    """Build the shared (replicated) and per-core input arrays."""
    bf = ml_dtypes.bfloat16
    nt, nr, d = emb_tables.shape
    tabs = np.ascontiguousarray(emb_tables.reshape(nt * nr, d)).astype(bf)

    def pad(a, shape):
        o = np.zeros(shape, np.float32)
        o[tuple(slice(0, s) for s in a.shape)] = a
        return o

    w0h = pad(np.asarray(bw0, np.float32).T, (16, 512)).astype(bf)
    w1h = pad(np.asarray(bw1, np.float32).T, (512, 256)).reshape(4, 128, 256)
    w1h = np.ascontiguousarray(w1h.transpose(1, 0, 2)).astype(bf)
    w2h = pad(np.asarray(bw2, np.float32).T, (256, 64)).reshape(2, 128, 64)
    w2h = np.ascontiguousarray(w2h.transpose(1, 0, 2)).astype(bf)
    tw0h = pad(np.asarray(tw0_, np.float32).T, (512, 512)).reshape(4, 128, 512)
    tw0h = np.ascontiguousarray(tw0h.transpose(1, 0, 2)).astype(bf)
    tw1h = pad(np.asarray(tw1_, np.float32).T, (512, 256)).reshape(4, 128, 256)
    tw1h = np.ascontiguousarray(tw1h.transpose(1, 0, 2)).astype(bf)
    tw2h = pad(np.asarray(tw2_, np.float32).T, (256, 1)).reshape(2, 128, 1)
    tw2h = np.ascontiguousarray(tw2h.transpose(1, 0, 2)).astype(bf)

    b0h = pad(np.asarray(bb0, np.float32), (512,)).reshape(4, 128).T.copy()
    b1h = pad(np.asarray(bb1, np.float32), (256,)).reshape(2, 128).T.copy()
    b2h = np.tile(pad(np.asarray(bb2, np.float32), (64,)), 2).reshape(128, 1).copy()
    tb0h = pad(np.asarray(tb0_, np.float32), (512,)).reshape(4, 128).T.copy()
    tb1h = pad(np.asarray(tb1_, np.float32), (256,)).reshape(2, 128).T.copy()
    tb2h = np.asarray(tb2_, np.float32).reshape(1, 1).copy()

    shared = dict(tabs=tabs, w0=w0h, b0=b0h, w1=w1h, b1=b1h, w2=w2h, b2=b2h,
                  tw0=tw0h, tb0=tb0h, tw1=tw1h, tb1=tb1h, tw2=tw2h, tb2=tb2h)

    # global row index = table*nr + idx; per-core sample-major [bc, NT]
    gflat = (np.arange(nt, dtype=np.int64)[:, None] * nr
             + np.asarray(indices, np.int64)).astype(np.int32)  # [NT, B]
    in_maps = []
    for c in range(n_cores):
        sl = slice(c * bc, (c + 1) * bc)
        gi = np.ascontiguousarray(gflat[:, sl].T)  # [bc, NT]
        dx = np.zeros((16, bc), np.float32)
        dx[:ND] = np.asarray(dense_x, np.float32)[sl].T
        in_maps.append(dict(shared, gidx=gi, dxt=dx.astype(bf)))
    return in_maps


_NC_CACHE = {}


def kernel(**inputs):
    key = "full"
    if key not in _NC_CACHE:
        _NC_CACHE[key] = build_kernel()
    nc = _NC_CACHE[key]
    in_maps = prep_host_inputs(**inputs)
    res = run_bass_kernel_spmd(nc, in_maps, core_ids=list(range(N_CORES)))
    outs = [res.results[c]["out"].reshape(BC) for c in range(N_CORES)]
    return np.concatenate(outs).reshape(B, 1).astype(np.float32)
